# revision 10
# baseline (speedup 1.0000x reference)
"""Depth-weighted average pooling (3x3, stride 2) on 8 Trainium2 NeuronCores.

out[n,c,ho,wo] = sum_ij x[n,c,2ho+i,2wo+j] * w_ij / sum_ij w_ij
  w_ij = exp(-|d[n,2ho+1,2wo+1] - d[n,2ho+i,2wo+j]|)

Sharding: batch N=8, one image per core (data parallel, no halo).

Per-core layout ("rows mod 4" mapping): SBUF partition p holds input rows
4p..4p+3 (tile dim t) plus a re-read of row 4p+4 (T0'), so partition p
computes output rows 2p ("even sub", input rows 4p..4p+2) and 2p+1
("odd sub", rows 4p+2..4p+4).  All engine access patterns start at
partition 0 (hardware requires start partition in {0,32,64,96}).
Weights are computed in the same [p, sub, wo] layout, so they are direct
per-partition operands of the vector engine — no cross-partition
broadcast is ever needed.  Channels live in the free dimension.
"""

import os
import sys
import functools

import numpy as np

for _p in ("/opt/trn_rl_repo", "/opt/trn_rl_repo/concourse"):
    if os.path.isdir(_p) and _p not in sys.path:
        sys.path.insert(0, _p)

KH = KW = 3
SH = SW = 2
N_CORES = 8
C, H, W = 64, 512, 512

# defaults used by kernel() and the test harness
DEFAULT_VARIANT = "bf16v2"
DEFAULT_G = 4

# taps in order; (1,1) is the center
TAPS = [(i, j) for i in range(3) for j in range(3)]
NC_TAPS = [t for t in TAPS if t != (1, 1)]


def AP_load5(x, c0, G, P, W):
    """DRAM view for [p, c, 5, w] <- x[c, 4p+t, w]: 5*W contiguous per (p,c)."""
    import concourse.bass as bass

    HW_ = x.shape[1] * x.shape[2]
    return bass.AP(
        x.tensor,
        c0 * HW_,
        [[4 * W, P], [HW_, G], [1, 5 * W]],
    )


def _main_bf16(nc, tc, xp, pp, ap_, x, o, wmb, C, G, W, Ho, Wo, Wp, PE, PO, ablate=""):
    """bf16 main channel loop.

    Per group: SWDGE cast-DMA loads x rows as contiguous bf16; ScalarE
    de-interleaves each row into three 4B-aligned tap arrays
    [j0 | j1 | j2] (each Wp wide) so every vector op runs in the packed
    2x bf16 mode; the odd-sub i=2 operand is built by a partition-shift
    SBUF->SBUF DMA of the expanded tile.  Accumulation in bf16; the
    store casts back to fp32 in the DMA.
    """
    from concourse import mybir

    bf16 = mybir.dt.bfloat16
    f32 = mybir.dt.float32

    preXC = None
    if "nodma" in ablate:
        preXC = ap_.tile([PE, G, 5, W], bf16, tag="preXC")
        nc.vector.memzero(preXC[:])
    for g in range(C // G):
        c0 = g * G
        if "nodma" in ablate:
            XC = preXC
        else:
            # main load rows 4p..4p+3 (8 KB DRAM reads) + t=4 re-read,
            # cast fp32->bf16 in the DMA (SWDGE ring carries only loads)
            XC = xp.tile([PE, G, 5, W], bf16, tag="XC")
            nc.gpsimd.dma_start(
                out=XC[0:PE, :, 0:4],
                in_=x[c0 : c0 + G, 0 : 4 * PE, :].rearrange(
                    "c (p t) w -> p c (t w)", t=4
                ),
            )
            nc.gpsimd.dma_start(
                out=XC[0:PO, :, 4],
                in_=x[c0 : c0 + G, 4 : 4 * PO + 1 : 4, :].transpose([1, 0, 2]),
            )
            if PE > PO:  # t=4 pad for the last partition
                nc.gpsimd.dma_start(
                    out=XC[PO:PE, :, 4],
                    in_=x[c0 : c0 + G, 0:1, :].unsqueeze(0),
                )
        # expand rows t=0..4: XB[p,c,t] = [x[..,0::2] | x[..,1::2] | x[..,2::2] pad]
        XB = xp.tile([PE, G, 5, 3 * Wp], bf16, tag="XB")
        nc.scalar.copy(XB[0:PE, :, :, 0:Wp], XC[0:PE, :, :, 0 : 2 * Wp : 2])
        nc.scalar.copy(XB[0:PE, :, :, Wp : 2 * Wp], XC[0:PE, :, :, 1 : 2 * Wp : 2])
        nc.scalar.copy(
            XB[0:PE, :, :, 2 * Wp : 2 * Wp + Wo],
            XC[0:PE, :, :, 2 : 2 + 2 * Wo : 2],
        )
        # defined (finite) pad column for j2
        nc.scalar.copy(XB[0:PE, :, :, 3 * Wp - 1 : 3 * Wp], XC[0:PE, :, :, 0:1])

        def xv_m(i, j):  # [PE, G, 2, Wp]; sub stride = 2 slots in t
            return XB[0:PE, :, i : i + 3 : 2, j * Wp : (j + 1) * Wp]

        def wv_m(wb):
            return wb[0:PE].unsqueeze(1).broadcast_to([PE, G, 2, Wp])

        acc = ap_.tile([PE, G, 2, Wp], bf16, tag="acc")
        if "nodve" in ablate:
            nc.vector.memzero(acc[:])
        else:
            nc.vector.tensor_mul(acc[:], xv_m(1, 1), wv_m(wmb[(1, 1)]))
            for (i, j) in NC_TAPS:
                p = pp.tile([PE, G, 2, Wp], bf16, tag="pm")
                nc.vector.tensor_mul(p[:], xv_m(i, j), wv_m(wmb[(i, j)]))
                nc.vector.tensor_add(acc[:], acc[:], p[:])

        if "nodma" in ablate:
            continue
        # convert to fp32 on ScalarE, then merged 2-row store on the ACT ring
        OS = ap_.tile([PE, G, 2, Wo], f32, tag="OS")
        nc.scalar.copy(OS[:], acc[0:PE, :, :, 0:Wo])
        nc.scalar.dma_start(
            out=o[c0 : c0 + G, 0 : 2 * PO, :].rearrange(
                "c (p s) w -> p c (s w)", s=2
            ),
            in_=OS[0:PO, :, :, :],
        )
        if Ho > 2 * PO:
            nc.scalar.dma_start(
                out=o[c0 : c0 + G, 2 * PO : Ho, :].transpose([1, 0, 2]),
                in_=OS[PO:PE, :, 0, :],
            )


def _main_bf16_v2(
    nc, tc, xp, pp, ap_, x, o, wmb, C, G, W, Ho, Wo, Wp, PE, PO, pool_adds=2,
    xcp=None
):
    """bf16 main loop, v2.

    - one 5-row SWDGE cast-load per group (10 KB DRAM descriptors) for
      partitions 0..PO-1; small tail DMAs for the last partition
    - ScalarE de-interleaves into packed tap arrays [j0 | j1 | j2]
    - DVE does the 9 tap products and most of the accumulation; the last
      `pool_adds` accumulate-adds run on GPSIMD (Pool) to unload DVE
    - SWDGE cast-store (bf16 acc -> fp32 DRAM) so ACT never touches the
      output path
    """
    import concourse.bass as bass
    from concourse import mybir

    bf16 = mybir.dt.bfloat16

    HW_ = x.shape[1] * x.shape[2]
    # split taps: DVE accumulates acc over TAPS_D; Pool folds TAPS_P in
    # (Pool does len(taps_p) adds total: fold-adds plus the final merge)
    taps_p = NC_TAPS[-pool_adds:] if pool_adds >= 1 else []
    taps_d = [t for t in NC_TAPS if t not in taps_p]

    for g in range(C // G):
        c0 = g * G
        XC = (xcp or xp).tile([PE, G, 5, W], bf16, tag="XC")
        # rows 4p..4p+4 as one contiguous 10 KB read per (p, c), cast to
        # bf16 in the DMA; overlapping row reads between partitions
        nc.gpsimd.dma_start(
            out=XC[0:PO, :, 0:5],
            in_=bass.AP(x.tensor, c0 * HW_, [[4 * W, PO], [HW_, G], [1, 5 * W]]),
        )
        if PE > PO:  # last partition: rows 4p..4p+3 + defined t=4 pad
            nc.gpsimd.dma_start(
                out=XC[PO:PE, :, 0:4],
                in_=x[c0 : c0 + G, 4 * PO : 4 * PE, :].rearrange(
                    "c (p t) w -> p c (t w)", t=4
                ),
            )
            nc.gpsimd.dma_start(
                out=XC[PO:PE, :, 4], in_=x[c0 : c0 + G, 0:1, :].unsqueeze(0)
            )
        # expand rows t=0..4: XB[p,c,t] = [x[..,0::2] | x[..,1::2] | x[..,2::2] pad]
        XB = xp.tile([PE, G, 5, 3 * Wp], bf16, tag="XB")
        nc.scalar.copy(XB[0:PE, :, :, 0:Wp], XC[0:PE, :, :, 0 : 2 * Wp : 2])
        nc.scalar.copy(XB[0:PE, :, :, Wp : 2 * Wp], XC[0:PE, :, :, 1 : 2 * Wp : 2])
        nc.scalar.copy(
            XB[0:PE, :, :, 2 * Wp : 2 * Wp + Wo],
            XC[0:PE, :, :, 2 : 2 + 2 * Wo : 2],
        )
        # defined (finite) pad column for j2
        nc.scalar.copy(XB[0:PE, :, :, 3 * Wp - 1 : 3 * Wp], XC[0:PE, :, :, 0:1])

        def xv_m(i, j):  # [PE, G, 2, Wp]; sub stride = 2 slots in t
            return XB[0:PE, :, i : i + 3 : 2, j * Wp : (j + 1) * Wp]

        def wv_m(wb):
            return wb[0:PE].unsqueeze(1).broadcast_to([PE, G, 2, Wp])

        # DVE chain: center product then taps_d accumulate
        acc = ap_.tile([PE, G, 2, Wp], bf16, tag="acc")
        nc.vector.tensor_mul(acc[:], xv_m(1, 1), wv_m(wmb[(1, 1)]))
        for (i, j) in taps_d:
            p = pp.tile([PE, G, 2, Wp], bf16, tag="pm")
            nc.vector.tensor_mul(p[:], xv_m(i, j), wv_m(wmb[(i, j)]))
            nc.vector.tensor_add(acc[:], acc[:], p[:])

        if taps_p:
            # Pool-side partial: first product written by DVE, remaining
            # products folded in by Pool, then Pool merges into acc
            accp = ap_.tile([PE, G, 2, Wp], bf16, tag="accp")
            (i0, j0) = taps_p[0]
            nc.vector.tensor_mul(accp[:], xv_m(i0, j0), wv_m(wmb[(i0, j0)]))
            for (i, j) in taps_p[1:]:
                q = pp.tile([PE, G, 2, Wp], bf16, tag="qm")
                nc.vector.tensor_mul(q[:], xv_m(i, j), wv_m(wmb[(i, j)]))
                nc.gpsimd.tensor_add(accp[:], accp[:], q[:])
            # packed [2, Wo] final tile so the store AP merges (s w) into one
            # contiguous run (DMA balancing needs <= 3 dims)
            out_t = ap_.tile([PE, G, 2, Wo], bf16, tag="accf")
            nc.gpsimd.tensor_add(
                out_t[:], acc[0:PE, :, :, 0:Wo], accp[0:PE, :, :, 0:Wo]
            )
        else:
            out_t = ap_.tile([PE, G, 2, Wo], bf16, tag="accf")
            nc.vector.tensor_copy(out_t[:], acc[0:PE, :, :, 0:Wo])

        # SWDGE cast-store: merged 2-row descriptors, bf16 -> fp32 in DMA
        nc.gpsimd.dma_start(
            out=o[c0 : c0 + G, 0 : 2 * PO, :].rearrange("c (p s) w -> p c (s w)", s=2),
            in_=out_t[0:PO, :, :, :],
        )
        if Ho > 2 * PO:
            nc.gpsimd.dma_start(
                out=o[c0 : c0 + G, 2 * PO : Ho, :].transpose([1, 0, 2]),
                in_=out_t[PO:PE, :, 0, :],
            )


def build_kernel(C=C, H=H, W=W, G=4, repeat=1, variant="fp32", ablate="",
                 pool_adds=None, xbufs=None):
    """Single-core Bass program: x[C,H,W], d[H,W] -> o[C,Ho,Wo].

    ablate: "nodve" skips the main-loop vector ops; "nodma" skips the
    x loads (compute reads whatever is resident).  For A/B timing only.
    """
    from contextlib import ExitStack

    import concourse.bacc as bacc
    from concourse.tile import TileContext
    from concourse import mybir

    f32 = mybir.dt.float32
    bf16 = mybir.dt.bfloat16
    AluOp = mybir.AluOpType
    Act = mybir.ActivationFunctionType
    Wp = W // 2  # padded output-width for bf16 tiles (= Wo+1, even)

    Ho = (H - KH) // SH + 1
    Wo = (W - KW) // SW + 1
    assert C % G == 0 and H % 4 == 0
    PE = (Ho + 1) // 2  # partitions carrying an even-sub output row
    PO = Ho // 2  # partitions carrying an odd-sub output row
    assert PE <= 128 and Ho == 2 * PE - 1

    nc = bacc.Bacc(
        "TRN2",
        target_bir_lowering=False,
        debug=False,
        enable_asserts=False,
        num_devices=1,
    )
    x = nc.dram_tensor("x", [C, H, W], f32, kind="ExternalInput").ap()
    d = nc.dram_tensor("d", [H, W], f32, kind="ExternalInput").ap()
    o = nc.dram_tensor("o", [C, Ho, Wo], f32, kind="ExternalOutput").ap()

    with TileContext(nc) as tc, ExitStack() as ctx:
        if pool_adds is None:
            pool_adds = {"bf16v2": 1, "bf16v3": 2}.get(variant, 1)
        if xbufs is None:
            xbufs = 3 if variant == "bf16v3" else 2
        xp = ctx.enter_context(tc.tile_pool(name="xp", bufs=2))
        xcp = None
        if xbufs > 2:
            xcp = ctx.enter_context(tc.tile_pool(name="xcp", bufs=xbufs))
        dp = ctx.enter_context(tc.tile_pool(name="dp", bufs=1))
        wp = ctx.enter_context(tc.tile_pool(name="wp", bufs=1))
        tp = ctx.enter_context(tc.tile_pool(name="tp", bufs=1))
        pp = ctx.enter_context(tc.tile_pool(name="pp", bufs=1))
        ap_ = ctx.enter_context(tc.tile_pool(name="ap", bufs=2))

        for _rep in range(repeat):
            # ---- depth tiles: DT[p, t, w] = d[4p+t, w]; D4[p, w] = d[4p+4, w]
            DT = dp.tile([PE, 4, W], f32, tag="DT")
            nc.sync.dma_start(
                out=DT[:], in_=d[0 : 4 * PE, :].rearrange("(p t) w -> p t w", t=4)
            )
            D4 = dp.tile([PO, W], f32, tag="D4")
            nc.sync.dma_start(out=D4[:], in_=d[4 : 4 * PO + 1 : 4, :])

            # window-center depth, both subs: dc[p, s, wo] = d[4p+2s+1, 2wo+1]
            dcm = DT[0:PE, 1:4:2, 1 : 1 + 2 * Wo : 2]  # [PE, 2, Wo]
            dce = DT[0:PE, 1, 1 : 1 + 2 * Wo : 2]  # [PE, Wo]
            dco = DT[0:PO, 3, 1 : 1 + 2 * Wo : 2]  # [PO, Wo]

            # ---- 8 non-center weight maps wm[p, s, wo] (normalized later)
            wm = {}
            for (i, j) in NC_TAPS:
                wt = wp.tile([PE, 2, Wo], f32, tag=f"w{i}{j}")
                if i < 2:
                    dv = DT[0:PE, i : i + 3 : 2, j : j + 2 * Wo : 2]
                    df = tp.tile([PE, 2, Wo], f32, tag="df")
                    nc.vector.tensor_sub(df[:], dcm, dv)
                    ab = tp.tile([PE, 2, Wo], f32, tag="ab")
                    nc.vector.scalar_tensor_tensor(
                        ab[:], df[:], -1.0, df[:], AluOp.mult, AluOp.max
                    )
                    nc.scalar.activation(wt[:], ab[:], Act.Exp, scale=-1.0)
                else:
                    # even sub from DT row t=2; odd sub from D4; pad rows -> 0
                    nc.vector.memzero(wt[:])
                    dfe = tp.tile([PE, Wo], f32, tag="dfe")
                    nc.vector.tensor_sub(dfe[:], dce, DT[0:PE, 2, j : j + 2 * Wo : 2])
                    abe = tp.tile([PE, Wo], f32, tag="abe")
                    nc.vector.scalar_tensor_tensor(
                        abe[:], dfe[:], -1.0, dfe[:], AluOp.mult, AluOp.max
                    )
                    nc.scalar.activation(wt[0:PE, 0, :], abe[:], Act.Exp, scale=-1.0)
                    dfo = tp.tile([PO, Wo], f32, tag="dfo")
                    nc.vector.tensor_sub(dfo[:], dco, D4[0:PO, j : j + 2 * Wo : 2])
                    abo = tp.tile([PO, Wo], f32, tag="abo")
                    nc.vector.scalar_tensor_tensor(
                        abo[:], dfo[:], -1.0, dfo[:], AluOp.mult, AluOp.max
                    )
                    nc.scalar.activation(wt[0:PO, 1, :], abo[:], Act.Exp, scale=-1.0)
                wm[(i, j)] = wt

            # ---- den = 1 + sum of the 8 maps; rden = 1/den
            # (Pool offload of this tree was tried and regressed: its in-order
            # queue serializes the TTs against SWDGE dispatch — keep on DVE)
            veng = nc.vector
            ks = list(wm)
            s01 = tp.tile([PE, 2, Wo], f32, tag="s01")
            veng.tensor_add(s01[:], wm[ks[0]][:], wm[ks[1]][:])
            s23 = tp.tile([PE, 2, Wo], f32, tag="s23")
            veng.tensor_add(s23[:], wm[ks[2]][:], wm[ks[3]][:])
            s45 = tp.tile([PE, 2, Wo], f32, tag="s45")
            veng.tensor_add(s45[:], wm[ks[4]][:], wm[ks[5]][:])
            s67 = tp.tile([PE, 2, Wo], f32, tag="s67")
            veng.tensor_add(s67[:], wm[ks[6]][:], wm[ks[7]][:])
            veng.tensor_add(s01[:], s01[:], s23[:])
            veng.tensor_add(s45[:], s45[:], s67[:])
            veng.tensor_add(s01[:], s01[:], s45[:])
            den = tp.tile([PE, 2, Wo], f32, tag="den")
            veng.tensor_scalar_add(den[:], s01[:], 1.0)
            rden = wp.tile([PE, 2, Wo], f32, tag="rden")
            nc.vector.reciprocal(rden[:], den[:])
            # normalize in place; center weight becomes rden itself
            for wt in wm.values():
                veng.tensor_mul(wt[:], wt[:], rden[:])

            if variant in ("bf16", "bf16v2", "bf16v3"):
                # convert the 9 normalized maps to padded bf16 tiles
                wmb = {}
                for (i, j) in NC_TAPS + [(1, 1)]:
                    src = rden if (i, j) == (1, 1) else wm[(i, j)]
                    wb = wp.tile([PE, 2, Wp], bf16, tag=f"wb{i}{j}")
                    nc.vector.memzero(wb[:])
                    nc.scalar.copy(wb[0:PE, :, 0:Wo], src[:])
                    wmb[(i, j)] = wb
                if variant == "bf16":
                    _main_bf16(
                        nc, tc, xp, pp, ap_, x, o, wmb, C, G, W, Ho, Wo, Wp, PE, PO
                    )
                else:
                    _main_bf16_v2(
                        nc, tc, xp, pp, ap_, x, o, wmb, C, G, W, Ho, Wo, Wp, PE, PO,
                        pool_adds=pool_adds, xcp=xcp,
                    )
                continue

            # ---- main channel loop
            preXT = None
            if "nodma" in ablate:
                preXT = ap_.tile([PE, G, 5, W], f32, tag="preXT")
                nc.vector.memzero(preXT[:])
            for g in range(C // G):
                c0 = g * G
                if "nodma" in ablate:
                    XT = preXT
                elif "shiftmode" not in ablate:
                    # main load: rows 4p..4p+3 = one 8 KB DRAM read per (p,c);
                    # t=4 (row 4p+4) re-read from DRAM as its own clean DMA
                    XT = xp.tile([PE, G, 5, W], f32, tag="XT")
                    nc.sync.dma_start(
                        out=XT[0:PE, :, 0:4],
                        in_=x[c0 : c0 + G, 0 : 4 * PE, :].rearrange(
                            "c (p t) w -> p c (t w)", t=4
                        ),
                    )
                    nc.sync.dma_start(
                        out=XT[0:PO, :, 4],
                        in_=x[c0 : c0 + G, 4 : 4 * PO + 1 : 4, :].transpose(
                            [1, 0, 2]
                        ),
                    )
                    if PE > PO:  # t=4 pad for the last partition
                        nc.sync.dma_start(
                            out=XT[PO:PE, :, 4],
                            in_=x[c0 : c0 + G, 0:1, :].unsqueeze(0),
                        )
                else:
                    # layout [p, c, t, w]: per (p,c) the 4 rows 4p..4p+3 are
                    # one contiguous 8 KB DRAM read
                    XT = xp.tile([PE, G, 5, W], f32, tag="XT")
                    nc.sync.dma_start(
                        out=XT[0:PE, :, 0:4],
                        in_=x[c0 : c0 + G, 0 : 4 * PE, :].rearrange(
                            "c (p t) w -> p c (t w)", t=4
                        ),
                    )
                    # t=4 = row 4p+4 = next partition's t=0, via DMA shift
                    # (SWDGE ring so it doesn't block later loads on SP)
                    if "noshift" not in ablate:
                        nc.gpsimd.dma_start(
                            out=XT[0:PO, :, 4], in_=XT[1 : PO + 1, :, 0]
                        )
                        if PE > PO:  # last partition's odd sub never stored
                            nc.gpsimd.dma_start(
                                out=XT[PO:PE, :, 4], in_=XT[0:1, :, 0]
                            )

                # x tap views, merged over subs (sub stride = 2 slots in t)
                def xv_m(i, j):  # [PE, G, 2, Wo]
                    return XT[0:PE, :, i : i + 3 : 2, j : j + 2 * Wo : 2]

                def wv_m(wt):  # [PE, G, 2, Wo] broadcast over channels
                    return wt[0:PE].unsqueeze(1).broadcast_to([PE, G, 2, Wo])

                acc = ap_.tile([PE, G, 2, Wo], f32, tag="acc")
                if "nodve" in ablate:
                    nc.vector.memzero(acc[:])
                else:
                    # center tap: acc = x_center * rden
                    nc.vector.tensor_mul(acc[:], xv_m(1, 1), wv_m(rden))
                    for (i, j) in NC_TAPS:
                        p = pp.tile([PE, G, 2, Wo], f32, tag="pm")
                        nc.vector.tensor_mul(p[:], xv_m(i, j), wv_m(wm[(i, j)]))
                        nc.vector.tensor_add(acc[:], acc[:], p[:])

                if "nodma" in ablate or "nostore" in ablate:
                    continue
                # ---- store: rows 2p,2p+1 together -> 2040B-contiguous DRAM
                # descriptors (ACT HWDGE ring so waits don't block SP loads)
                nc.scalar.dma_start(
                    out=o[c0 : c0 + G, 0 : 2 * PO, :].rearrange(
                        "c (p s) w -> p c (s w)", s=2
                    ),
                    in_=acc[0:PO, :, :, :],
                )
                if Ho > 2 * PO:
                    nc.scalar.dma_start(  # last even row (ho = 2*PO)
                        out=o[c0 : c0 + G, 2 * PO : Ho, :].transpose([1, 0, 2]),
                        in_=acc[PO:PE, :, 0, :],
                    )

    nc.compile()
    return nc


@functools.lru_cache(maxsize=4)
def _compiled(key):
    C_, H_, W_, G, repeat, variant = key
    return build_kernel(C=C_, H=H_, W=W_, G=G, repeat=repeat, variant=variant)


def kernel(input, depth):
    """Full-io entry: input [8,64,512,512] f32, depth [8,1,512,512] f32."""
    from concourse import bass_utils

    input = np.ascontiguousarray(np.asarray(input), dtype=np.float32)
    depth = np.ascontiguousarray(np.asarray(depth), dtype=np.float32)
    N = input.shape[0]
    assert N == N_CORES and input.shape[1:] == (C, H, W)

    nc = _compiled((C, H, W, DEFAULT_G, 1, DEFAULT_VARIANT))
    in_maps = [{"x": input[n], "d": depth[n, 0]} for n in range(N)]
    res = bass_utils.run_bass_kernel_spmd(nc, in_maps, core_ids=list(range(N)))
    out = np.stack([r["o"] for r in res.results], axis=0)
    return out


if __name__ == "__main__":
    nc = build_kernel()
    print("built ok")

